# revision 1
# baseline (speedup 1.0000x reference)
"""MoE (dense-act-dense, top-4 of 8 experts) Trainium2 kernel.

Strategy (expert-parallel, host-side dispatch):
  - The forward combine weight is exactly 1.0 (straight-through gate trick in
    the reference), so out[n] = sum_{e in top4(n)} expert_e(x[n]).
  - Host computes the tiny gate matmul + top-4 routing (0.05% of FLOPs) and
    dispatches tokens: core e receives the tokens routed to expert e
    (capacity-padded), plus expert e's weights. This is the sharding step.
  - Each of the 8 cores runs a dense 2-layer MLP (relu between) on its tokens:
      h = relu(w1[e] @ x) ; y = w2[e] @ h
    as two chained fp32r GEMMs (fp32 data, FP22 multiply, fp32 accumulate).
  - Host scatter-adds per-expert outputs back (weight 1.0 per selection).

Per-core device layouts (everything pre-transposed on host for contiguous DMA):
  xT  [D, C] f32r : routed tokens, transposed
  w1t [D, H] f32r : w1[e].T
  w2t [H, O] f32r : w2[e].T
  yT  [O, C] f32  : expert output, transposed

Schedule notes:
  - Capacity is exact (max expert load, even-rounded), split into even tile
    widths in [256, 512]: fp32r requires even moving counts; measured per-mm
    spacing is ~(NT+32..44)cyc so per-token cost is flat for NT in 320..460.
  - Weights are DMAed in 128-wide column slices (separate tiles) so GEMM
    chains start as soon as their slice lands instead of after the full 8MB.
  - DMA emission order on the sync queue is hand-tuned: x0, w1 slices, x1,
    w2[0:8], x2, w2[8:16], x3, ... so the PE's program-order needs roughly
    track the FIFO queue's delivery order during the ~26MB startup stream.
  - GEMM2(t) is emitted one tile behind GEMM1(t+1) (depth-1 software
    pipeline) to give the PE GEMM1 work while w2 is still streaming in.
  - y drains: PSUM -> SBUF copy on vector, store DMA issued on scalar, so the
    sync queue (x + weights, latency-critical) is never blocked behind them.
"""

import numpy as np
from contextlib import ExitStack

import concourse.bass as bass
import concourse.tile as tile
from concourse import bacc, mybir
from concourse import bass_utils

F32 = mybir.dt.float32
F32R = mybir.dt.float32r
P = 128

TOP_K = 4
D, H, O, E = 2048, 1024, 2048, 8
_NC_CACHE = {}


def _tile_widths(C, target):
    """Split C tokens (padded to even) into even tiles of near-equal width in
    [256, 512]. Even widths are an fp32r ISA requirement; >=256 keeps fp32r at
    1 cycle/row; wider tiles amortize the fixed ~32-cycle per-matmul bubble."""
    C = max(C + (C % 2), 256)
    C2 = C // 2
    ntiles = min(-(-C // target), C2 // 128)
    base = C2 // ntiles
    rem = C2 - base * ntiles
    widths = [2 * (base + 1)] * rem + [2 * base] * (ntiles - rem)
    widths.sort(reverse=True)
    assert sum(widths) == C and all(256 <= w <= 512 and w % 2 == 0 for w in widths)
    return widths


def build_expert_kernel(C, target):
    """Per-core program: dense [C, D] @ [D, H] -> relu -> @ [H, O] in fp32r."""
    DC, HC, OC = D // P, H // P, O // P
    widths = _tile_widths(C, target)
    starts = [sum(widths[:i]) for i in range(len(widths))]
    NTILES = len(widths)
    NTMAX = max(widths)
    nc = bacc.Bacc("TRN2", target_bir_lowering=False, debug=False, num_devices=E)
    xT = nc.dram_tensor("xT", [D, C], F32R, kind="ExternalInput").ap()
    w1t = nc.dram_tensor("w1t", [D, H], F32R, kind="ExternalInput").ap()
    w2t = nc.dram_tensor("w2t", [H, O], F32R, kind="ExternalInput").ap()
    yT = nc.dram_tensor("yT", [O, C], F32, kind="ExternalOutput").ap()

    with tile.TileContext(nc) as tc, ExitStack() as ctx:
        wpool = ctx.enter_context(tc.tile_pool(name="w", bufs=1))
        xpool = ctx.enter_context(tc.tile_pool(name="x", bufs=2))
        hpool = ctx.enter_context(tc.tile_pool(name="h", bufs=2))
        ypool = ctx.enter_context(tc.tile_pool(name="y", bufs=4))
        ps1 = ctx.enter_context(tc.tile_pool(name="ps1", bufs=2, space="PSUM"))
        ps2 = ctx.enter_context(tc.tile_pool(name="ps2", bufs=4, space="PSUM"))

        x_tiles = {}

        def dma_x(t):
            w_t = widths[t]
            x_t = xpool.tile([P, DC, NTMAX], F32R, name="x_t")[:, :, :w_t]
            nc.sync.dma_start(
                x_t[:],
                xT[:, starts[t]:starts[t] + w_t].rearrange("(dc p) n -> p dc n", p=P),
            )
            x_tiles[t] = x_t

        # --- startup DMA stream, hand-ordered for the FIFO queue ---
        dma_x(0)
        w1s = []
        for hc in range(HC):
            w = wpool.tile([P, DC, P], F32R, name=f"w1s{hc}")
            nc.sync.dma_start(
                w[:],
                w1t[:, hc * P:(hc + 1) * P].rearrange("(dc p) h -> p dc h", p=P),
            )
            w1s.append(w)
        if NTILES > 1:
            dma_x(1)
        w2s = []

        def dma_w2(oc):
            w = wpool.tile([P, HC, P], F32R, name=f"w2s{oc}")
            nc.sync.dma_start(
                w[:],
                w2t[:, oc * P:(oc + 1) * P].rearrange("(hc p) o -> p hc o", p=P),
            )
            w2s.append(w)

        for oc in range(OC // 2):
            dma_w2(oc)

        def gemm1(t):
            w_t = widths[t]
            x_t = x_tiles.pop(t)
            h_t = hpool.tile([P, HC, NTMAX], F32R, name="h_t")[:, :, :w_t]
            for hc in range(HC):
                ph = ps1.tile([P, NTMAX], F32, name="ph")[:, :w_t]
                for dc in range(DC):
                    nc.tensor.matmul(
                        ph[:], w1s[hc][:, dc, :], x_t[:, dc, :],
                        start=(dc == 0), stop=(dc == DC - 1),
                    )
                nc.scalar.activation(
                    h_t[:, hc, :], ph[:], mybir.ActivationFunctionType.Relu
                )
            return h_t

        def gemm2(t, h_t):
            w_t = widths[t]
            for oc in range(OC):
                po = ps2.tile([P, NTMAX], F32, name="po")[:, :w_t]
                for hc in range(HC):
                    nc.tensor.matmul(
                        po[:], w2s[oc][:, hc, :], h_t[:, hc, :],
                        start=(hc == 0), stop=(hc == HC - 1),
                    )
                y_t = ypool.tile([P, NTMAX], F32, name="y_t")[:, :w_t]
                nc.vector.tensor_copy(y_t[:], po[:])
                nc.scalar.dma_start(
                    yT[oc * P:(oc + 1) * P, starts[t]:starts[t] + w_t], y_t[:]
                )

        # --- depth-1 software-pipelined main loop: GEMM2 runs one tile
        # behind GEMM1 so the PE has work while w2 streams in at startup ---
        h_tiles = {}
        for t in range(NTILES):
            if t + 1 < NTILES and t >= 1:
                dma_x(t + 1)
            h_tiles[t] = gemm1(t)
            if t == 1:
                for oc in range(OC // 2, OC):
                    dma_w2(oc)
            if t >= 1:
                gemm2(t - 1, h_tiles.pop(t - 1))
        gemm2(NTILES - 1, h_tiles.pop(NTILES - 1))
    nc.compile()
    return nc


def _route(xt, wg):
    """Host-side gate + top-4. Gap between 4th/5th gate values is ~3e-5 for
    this distribution, far above fp32 matmul noise, so fp32 reproduces the
    reference top-k set exactly."""
    gate = xt @ wg  # [N, E] fp32
    top4 = np.argpartition(-gate, TOP_K - 1, axis=1)[:, :TOP_K]  # set, unordered
    return top4


def kernel(x, wg, w1, w2, _want_results=False, _run_kwargs=None):
    x = np.asarray(x, dtype=np.float32)
    wg = np.asarray(wg, dtype=np.float32)
    w1 = np.asarray(w1, dtype=np.float32)
    w2 = np.asarray(w2, dtype=np.float32)
    B, S, Dx = x.shape
    N = B * S
    xt = np.ascontiguousarray(x.reshape(N, Dx))
    top4 = _route(xt, wg)

    # token lists per expert
    sel = np.zeros((N, E), dtype=bool)
    np.put_along_axis(sel, top4, True, axis=1)
    tokens = [np.nonzero(sel[:, e])[0] for e in range(E)]
    counts = np.array([len(t) for t in tokens])
    CAP = max(int(counts.max()), 256)
    CAP += CAP % 2

    if CAP not in _NC_CACHE:
        # Wider tiles amortize the per-matmul bubble best, but the widest
        # config cuts SBUF very close — fall back to narrower tiles if the
        # allocator rejects it.
        last_err = None
        for target in (384, 352, 320):
            try:
                _NC_CACHE[CAP] = build_expert_kernel(CAP, target)
                break
            except ValueError as err:  # SBUF pool allocation failure
                last_err = err
        else:
            raise last_err
    nc = _NC_CACHE[CAP]

    in_maps = []
    for e in range(E):
        xe = np.zeros((CAP, Dx), dtype=np.float32)
        xe[:counts[e]] = xt[tokens[e]]
        in_maps.append({
            "xT": np.ascontiguousarray(xe.T),
            "w1t": np.ascontiguousarray(w1[e].T),
            "w2t": np.ascontiguousarray(w2[e].T),
        })

    res = bass_utils.run_bass_kernel_spmd(
        nc, in_maps, core_ids=list(range(E)), **(_run_kwargs or {})
    )

    out = np.zeros((N, O), dtype=np.float32)
    for e in range(E):
        out[tokens[e]] += res.results[e]["yT"].T[:counts[e]]
    out = out.reshape(B, S, O)
    if _want_results:
        return out, res
    return out



# revision 2
# speedup vs baseline: 1.1202x; 1.1202x over previous
"""MoE (dense-act-dense, top-4 of 8 experts) Trainium2 kernel.

Strategy (expert-parallel, host-side dispatch):
  - The forward combine weight is exactly 1.0 (straight-through gate trick in
    the reference), so out[n] = sum_{e in top4(n)} expert_e(x[n]).
  - Host computes the tiny gate matmul + top-4 routing (0.05% of FLOPs) and
    dispatches tokens: core e receives the tokens routed to expert e
    (capacity-padded), plus expert e's weights. This is the sharding step.
  - Each of the 8 cores runs a dense 2-layer MLP (relu between) on its tokens:
      h = relu(w1[e] @ x) ; y = w2[e] @ h
    as two chained bf16 GEMMs (bf16 data, fp32 PSUM accumulate). bf16 halves
    DMA traffic + SBUF vs fp32r at the same 1 cycle/row PE rate, and its
    ~3e-3 rel-err is far inside the 2e-2 gate.
  - Host scatter-adds per-expert outputs back (weight 1.0 per selection).

Per-core device layouts (everything pre-transposed on host for contiguous DMA):
  xT  [D, C] bf16 : routed tokens, transposed
  w1t [D, H] bf16 : w1[e].T
  w2t [H, O] bf16 : w2[e].T
  yT  [O, C] f32  : expert output, transposed

Schedule notes:
  - Capacity is exact (max expert load, even-rounded), split into even tile
    widths <= 512 (PSUM bank limit): measured per-mm spacing is ~(NT+40)cyc,
    so the widest legal tiles minimize the fixed per-matmul bubble.
  - Weights are DMAed in 256-wide column slices (512B/partition descriptors,
    the efficient-DMA threshold) so GEMM chains start as soon as their slice
    lands instead of after the full 4MB.
  - DMA emission order on the sync queue: x0, w1 slices, x1, all w2 slices,
    x2 — at 358GB/s the whole 25MB input stream lands by ~70us, well before
    gemm2(0) needs w2 (~70us) or any gemm1 needs its x tile.
  - GEMM2(t) is emitted one tile behind GEMM1(t+1) (depth-1 software
    pipeline) to give the PE GEMM1 work while w2 is still streaming in.
  - y drains: PSUM -> SBUF copy on vector, store DMA issued on scalar, so the
    sync queue (x + weights, latency-critical) is never blocked behind them.
"""

import numpy as np
import ml_dtypes
from contextlib import ExitStack

import concourse.bass as bass
import concourse.tile as tile
from concourse import bacc, mybir
from concourse import bass_utils

F32 = mybir.dt.float32
BF16 = mybir.dt.bfloat16
P = 128

TOP_K = 4
D, H, O, E = 2048, 1024, 2048, 8
_NC_CACHE = {}
NPBF16 = ml_dtypes.bfloat16


def _tile_widths(C, target=512):
    """Split C tokens (padded to even) into even tiles of near-equal width
    <= target (PSUM bank holds 512 fp32). Wider tiles amortize the fixed
    ~40-cycle per-matmul bubble."""
    C = max(C + (C % 2), 256)
    C2 = C // 2
    ntiles = -(-C // target)
    base = C2 // ntiles
    rem = C2 - base * ntiles
    widths = [2 * (base + 1)] * rem + [2 * base] * (ntiles - rem)
    widths.sort(reverse=True)
    assert sum(widths) == C and all(w <= target and w % 2 == 0 for w in widths)
    return widths


def build_expert_kernel(C, target=512):
    """Per-core program: dense [C, D] @ [D, H] -> relu -> @ [H, O] in bf16."""
    DC, HC, OC = D // P, H // P, O // P
    widths = _tile_widths(C, target)
    starts = [sum(widths[:i]) for i in range(len(widths))]
    NTILES = len(widths)
    NTMAX = max(widths)
    nc = bacc.Bacc("TRN2", target_bir_lowering=False, debug=False, num_devices=E)
    xT = nc.dram_tensor("xT", [D, C], BF16, kind="ExternalInput").ap()
    w1t = nc.dram_tensor("w1t", [D, H], BF16, kind="ExternalInput").ap()
    w2t = nc.dram_tensor("w2t", [H, O], BF16, kind="ExternalInput").ap()
    yT = nc.dram_tensor("yT", [O, C], F32, kind="ExternalOutput").ap()

    with tile.TileContext(nc) as tc, ExitStack() as ctx:
        wpool = ctx.enter_context(tc.tile_pool(name="w", bufs=1))
        xpool = ctx.enter_context(tc.tile_pool(name="x", bufs=3))
        hpool = ctx.enter_context(tc.tile_pool(name="h", bufs=2))
        ypool = ctx.enter_context(tc.tile_pool(name="y", bufs=4))
        ps1 = ctx.enter_context(tc.tile_pool(name="ps1", bufs=2, space="PSUM"))
        ps2 = ctx.enter_context(tc.tile_pool(name="ps2", bufs=4, space="PSUM"))

        x_tiles = {}

        def dma_x(t):
            w_t = widths[t]
            x_t = xpool.tile([P, DC, NTMAX], BF16, name="x_t")[:, :, :w_t]
            nc.sync.dma_start(
                x_t[:],
                xT[:, starts[t]:starts[t] + w_t].rearrange("(dc p) n -> p dc n", p=P),
            )
            x_tiles[t] = x_t

        # --- startup DMA stream, hand-ordered for the FIFO queue ---
        dma_x(0)
        w1s = []
        for j in range(H // 256):
            w = wpool.tile([P, DC, 256], BF16, name=f"w1s{j}")
            nc.sync.dma_start(
                w[:],
                w1t[:, j * 256:(j + 1) * 256].rearrange("(dc p) h -> p dc h", p=P),
            )
            w1s.append(w)
        if NTILES > 1:
            dma_x(1)
        w2s = []
        for j in range(O // 256):
            w = wpool.tile([P, HC, 256], BF16, name=f"w2s{j}")
            nc.sync.dma_start(
                w[:],
                w2t[:, j * 256:(j + 1) * 256].rearrange("(hc p) o -> p hc o", p=P),
            )
            w2s.append(w)
        if NTILES > 2:
            dma_x(2)

        def gemm1(t):
            w_t = widths[t]
            x_t = x_tiles.pop(t)
            h_t = hpool.tile([P, HC, NTMAX], BF16, name="h_t")[:, :, :w_t]
            for hc in range(HC):
                ph = ps1.tile([P, NTMAX], F32, name="ph")[:, :w_t]
                w1w = w1s[hc // 2][:, :, (hc % 2) * P:(hc % 2) * P + P]
                for dc in range(DC):
                    nc.tensor.matmul(
                        ph[:], w1w[:, dc, :], x_t[:, dc, :],
                        start=(dc == 0), stop=(dc == DC - 1),
                    )
                nc.scalar.activation(
                    h_t[:, hc, :], ph[:], mybir.ActivationFunctionType.Relu
                )
            return h_t

        def gemm2(t, h_t):
            w_t = widths[t]
            for oc in range(OC):
                po = ps2.tile([P, NTMAX], F32, name="po")[:, :w_t]
                w2w = w2s[oc // 2][:, :, (oc % 2) * P:(oc % 2) * P + P]
                for hc in range(HC):
                    nc.tensor.matmul(
                        po[:], w2w[:, hc, :], h_t[:, hc, :],
                        start=(hc == 0), stop=(hc == HC - 1),
                    )
                y_t = ypool.tile([P, NTMAX], F32, name="y_t")[:, :w_t]
                nc.vector.tensor_copy(y_t[:], po[:])
                nc.scalar.dma_start(
                    yT[oc * P:(oc + 1) * P, starts[t]:starts[t] + w_t], y_t[:]
                )

        # --- depth-1 software-pipelined main loop: GEMM2 runs one tile
        # behind GEMM1 so the PE has work while w2 streams in at startup ---
        h_tiles = {}
        for t in range(NTILES):
            h_tiles[t] = gemm1(t)
            if t >= 1:
                gemm2(t - 1, h_tiles.pop(t - 1))
            if t + 3 < NTILES:
                dma_x(t + 3)
        gemm2(NTILES - 1, h_tiles.pop(NTILES - 1))
    nc.compile()
    return nc


def _route(xt, wg):
    """Host-side gate + top-4. Gap between 4th/5th gate values is ~3e-5 for
    this distribution, far above fp32 matmul noise, so fp32 reproduces the
    reference top-k set exactly."""
    gate = xt @ wg  # [N, E] fp32
    top4 = np.argpartition(-gate, TOP_K - 1, axis=1)[:, :TOP_K]  # set, unordered
    return top4


def kernel(x, wg, w1, w2, _want_results=False, _run_kwargs=None):
    x = np.asarray(x, dtype=np.float32)
    wg = np.asarray(wg, dtype=np.float32)
    w1 = np.asarray(w1, dtype=np.float32)
    w2 = np.asarray(w2, dtype=np.float32)
    B, S, Dx = x.shape
    N = B * S
    xt = np.ascontiguousarray(x.reshape(N, Dx))
    top4 = _route(xt, wg)

    # token lists per expert
    sel = np.zeros((N, E), dtype=bool)
    np.put_along_axis(sel, top4, True, axis=1)
    tokens = [np.nonzero(sel[:, e])[0] for e in range(E)]
    counts = np.array([len(t) for t in tokens])
    CAP = max(int(counts.max()), 256)
    CAP += CAP % 2

    if CAP not in _NC_CACHE:
        _NC_CACHE[CAP] = build_expert_kernel(CAP)
    nc = _NC_CACHE[CAP]

    xbf = xt.astype(NPBF16)
    in_maps = []
    for e in range(E):
        xe = np.zeros((CAP, Dx), dtype=NPBF16)
        xe[:counts[e]] = xbf[tokens[e]]
        in_maps.append({
            "xT": np.ascontiguousarray(xe.T),
            "w1t": np.ascontiguousarray(w1[e].T.astype(NPBF16)),
            "w2t": np.ascontiguousarray(w2[e].T.astype(NPBF16)),
        })

    res = bass_utils.run_bass_kernel_spmd(
        nc, in_maps, core_ids=list(range(E)), **(_run_kwargs or {})
    )

    out = np.zeros((N, O), dtype=np.float32)
    for e in range(E):
        out[tokens[e]] += res.results[e]["yT"].T[:counts[e]]
    out = out.reshape(B, S, O)
    if _want_results:
        return out, res
    return out


# revision 3
# speedup vs baseline: 1.1234x; 1.0028x over previous
"""MoE (dense-act-dense, top-4 of 8 experts) Trainium2 kernel.

Strategy (expert-parallel, host-side dispatch):
  - The forward combine weight is exactly 1.0 (straight-through gate trick in
    the reference), so out[n] = sum_{e in top4(n)} expert_e(x[n]).
  - Host computes the tiny gate matmul + top-4 routing (0.05% of FLOPs) and
    dispatches tokens: core e receives the tokens routed to expert e
    (capacity-padded), plus expert e's weights. This is the sharding step.
  - Each of the 8 cores runs a dense 2-layer MLP (relu between) on its tokens:
      h = relu(w1[e] @ x) ; y = w2[e] @ h
    as two chained bf16 GEMMs (bf16 data, fp32 PSUM accumulate). bf16 halves
    DMA traffic + SBUF vs fp32r at the same 1 cycle/row PE rate, and its
    ~3e-3 rel-err is far inside the 2e-2 gate.
  - Host scatter-adds per-expert outputs back (weight 1.0 per selection).

Per-core device layouts (host pre-arranges everything for contiguous DMA):
  xT  [D, C]  bf16 : routed tokens, transposed (924B runs per partition)
  w1r [H, D]  bf16 : slice-major stationary layout; rows hc*128+k hold
                     w1[e][hc*128+m, dc*128+k] at col dc*128+m, so a 128-col
                     PE slice DMAs as one 4KB-contiguous run per partition.
  w2r [O, H]  bf16 : same trick for layer 2 (2KB runs).
  yT  [O, C]  f32  : expert output, transposed.

Schedule notes:
  - Capacity is exact (max expert load, even-rounded), split into even tile
    widths <= 512 (PSUM bank limit): measured per-mm spacing is ~(NT+10)cyc
    in bf16, so the widest legal tiles minimize the fixed bubble.
  - ~44 dummy matmuls on a memset scratch tile run during the ~12us DMA
    startup window so the PE p-state is fully ramped (2.4GHz) before real
    work; without this the first ~10 real matmuls run at ~half speed.
  - x tiles stream in 4 dc-chunks so the first gemm1 chain starts after
    ~1MB (w1 slice 0 + x0 chunk 0) instead of the full 2.9MB.
  - DMA order on the sync queue: w1s0, x0 (4 chunks), w1s1..7, x1, w2s0..15,
    x2, then one x tile per loop iteration. At ~300GB/s observed, every
    consumer's data lands just ahead of the PE's zero-gap schedule.
  - GEMM2(t) is emitted one tile behind GEMM1(t+1) (depth-1 software
    pipeline) to give the PE GEMM1 work while w2 is still streaming in.
  - y drains: PSUM -> SBUF copy on vector, store DMA issued on scalar, so the
    sync queue (x + weights, latency-critical) is never blocked behind them.
"""

import numpy as np
import ml_dtypes
from contextlib import ExitStack

import concourse.bass as bass
import concourse.tile as tile
from concourse import bacc, mybir
from concourse import bass_utils

F32 = mybir.dt.float32
BF16 = mybir.dt.bfloat16
P = 128

TOP_K = 4
D, H, O, E = 2048, 1024, 2048, 8
_NC_CACHE = {}
NPBF16 = ml_dtypes.bfloat16


def _tile_widths(C, target=512):
    """Split C tokens (padded to even) into even tiles of near-equal width
    <= target (PSUM bank holds 512 fp32)."""
    C = max(C + (C % 2), 256)
    C2 = C // 2
    ntiles = -(-C // target)
    base = C2 // ntiles
    rem = C2 - base * ntiles
    widths = [2 * (base + 1)] * rem + [2 * base] * (ntiles - rem)
    widths.sort(reverse=True)
    assert sum(widths) == C and all(w <= target and w % 2 == 0 for w in widths)
    return widths


def build_expert_kernel(C, target=512):
    """Per-core program: dense [C, D] @ [D, H] -> relu -> @ [H, O] in bf16."""
    DC, HC, OC = D // P, H // P, O // P
    XG = 4  # dc-groups per x tile (chunked DMA)
    widths = _tile_widths(C, target)
    starts = [sum(widths[:i]) for i in range(len(widths))]
    NTILES = len(widths)
    NTMAX = max(widths)
    nc = bacc.Bacc("TRN2", target_bir_lowering=False, debug=False, num_devices=E)
    xT = nc.dram_tensor("xT", [D, C], BF16, kind="ExternalInput").ap()
    w1r = nc.dram_tensor("w1r", [H, D], BF16, kind="ExternalInput").ap()
    w2r = nc.dram_tensor("w2r", [O, H], BF16, kind="ExternalInput").ap()
    yT = nc.dram_tensor("yT", [O, C], F32, kind="ExternalOutput").ap()

    with tile.TileContext(nc) as tc, ExitStack() as ctx:
        dpool = ctx.enter_context(tc.tile_pool(name="d", bufs=1))
        wpool = ctx.enter_context(tc.tile_pool(name="w", bufs=1))
        xpool = ctx.enter_context(tc.tile_pool(name="x", bufs=3 * XG))
        hpool = ctx.enter_context(tc.tile_pool(name="h", bufs=2))
        ypool = ctx.enter_context(tc.tile_pool(name="y", bufs=4))
        psd = ctx.enter_context(tc.tile_pool(name="psd", bufs=1, space="PSUM"))
        ps1 = ctx.enter_context(tc.tile_pool(name="ps1", bufs=2, space="PSUM"))
        ps2 = ctx.enter_context(tc.tile_pool(name="ps2", bufs=4, space="PSUM"))

        # --- PE p-state warmup: ~44 dummy matmuls on a zeroed scratch tile
        # fill the DMA startup window so real matmuls start at full clock ---
        dum = dpool.tile([P, 512], BF16, name="dum")
        nc.vector.memset(dum[:], 0.0)
        pd = psd.tile([P, 512], F32, name="pd")
        NWARM = 44
        for i in range(NWARM):
            nc.tensor.matmul(
                pd[:], dum[:, 0:P], dum[:],
                start=(i % 11 == 0), stop=(i % 11 == 10),
            )

        x_tiles = {}

        def dma_x(t):
            w_t = widths[t]
            chunks = []
            for g in range(XG):
                xc = xpool.tile([P, DC // XG, NTMAX], BF16, name="x_t")[:, :, :w_t]
                nc.sync.dma_start(
                    xc[:],
                    xT[g * (D // XG):(g + 1) * (D // XG),
                       starts[t]:starts[t] + w_t].rearrange(
                        "(dc p) n -> p dc n", p=P),
                )
                chunks.append(xc)
            x_tiles[t] = chunks

        w1s = [None] * HC

        def dma_w1(hc):
            w = wpool.tile([P, DC, P], BF16, name=f"w1s{hc}")
            nc.sync.dma_start(
                w[:],
                w1r[hc * P:(hc + 1) * P, :].rearrange("p (dc j) -> p dc j", dc=DC),
            )
            w1s[hc] = w

        w2s = [None] * OC

        def dma_w2(oc):
            w = wpool.tile([P, HC, P], BF16, name=f"w2s{oc}")
            nc.sync.dma_start(
                w[:],
                w2r[oc * P:(oc + 1) * P, :].rearrange("p (hc j) -> p hc j", hc=HC),
            )
            w2s[oc] = w

        # --- startup DMA stream, hand-ordered for the FIFO queue ---
        dma_w1(0)
        dma_x(0)
        for hc in range(1, HC):
            dma_w1(hc)
        if NTILES > 1:
            dma_x(1)
        for oc in range(OC):
            dma_w2(oc)
        if NTILES > 2:
            dma_x(2)

        def gemm1(t):
            w_t = widths[t]
            xc = x_tiles.pop(t)
            h_t = hpool.tile([P, HC, NTMAX], BF16, name="h_t")[:, :, :w_t]
            for hc in range(HC):
                ph = ps1.tile([P, NTMAX], F32, name="ph")[:, :w_t]
                for dc in range(DC):
                    nc.tensor.matmul(
                        ph[:], w1s[hc][:, dc, :], xc[dc // XG][:, dc % XG, :],
                        start=(dc == 0), stop=(dc == DC - 1),
                    )
                nc.scalar.activation(
                    h_t[:, hc, :], ph[:], mybir.ActivationFunctionType.Relu
                )
            return h_t

        def gemm2(t, h_t):
            w_t = widths[t]
            for oc in range(OC):
                po = ps2.tile([P, NTMAX], F32, name="po")[:, :w_t]
                for hc in range(HC):
                    nc.tensor.matmul(
                        po[:], w2s[oc][:, hc, :], h_t[:, hc, :],
                        start=(hc == 0), stop=(hc == HC - 1),
                    )
                y_t = ypool.tile([P, NTMAX], F32, name="y_t")[:, :w_t]
                nc.vector.tensor_copy(y_t[:], po[:])
                nc.scalar.dma_start(
                    yT[oc * P:(oc + 1) * P, starts[t]:starts[t] + w_t], y_t[:]
                )

        # --- depth-1 software-pipelined main loop: GEMM2 runs one tile
        # behind GEMM1 so the PE has work while w2 streams in at startup ---
        h_tiles = {}
        for t in range(NTILES):
            h_tiles[t] = gemm1(t)
            if t >= 1:
                gemm2(t - 1, h_tiles.pop(t - 1))
            if t + 3 < NTILES:
                dma_x(t + 3)
        gemm2(NTILES - 1, h_tiles.pop(NTILES - 1))
    nc.compile()
    return nc


def _route(xt, wg):
    """Host-side gate + top-4. Gap between 4th/5th gate values is ~3e-5 for
    this distribution, far above fp32 matmul noise, so fp32 reproduces the
    reference top-k set exactly."""
    gate = xt @ wg  # [N, E] fp32
    top4 = np.argpartition(-gate, TOP_K - 1, axis=1)[:, :TOP_K]  # set, unordered
    return top4


def _w1_slice_major(w1e):
    """[H, D] -> rows hc*128+k, cols dc*128+m = w1e[hc*128+m, dc*128+k]."""
    HC, DC = H // P, D // P
    return np.ascontiguousarray(
        w1e.reshape(HC, P, DC, P).transpose(0, 3, 2, 1).reshape(H, D)
    )


def _w2_slice_major(w2e):
    """[O, H] -> rows oc*128+k, cols hc*128+m = w2e[oc*128+m, hc*128+k]."""
    OC, HC = O // P, H // P
    return np.ascontiguousarray(
        w2e.reshape(OC, P, HC, P).transpose(0, 3, 2, 1).reshape(O, H)
    )


def kernel(x, wg, w1, w2, _want_results=False, _run_kwargs=None):
    x = np.asarray(x, dtype=np.float32)
    wg = np.asarray(wg, dtype=np.float32)
    w1 = np.asarray(w1, dtype=np.float32)
    w2 = np.asarray(w2, dtype=np.float32)
    B, S, Dx = x.shape
    N = B * S
    xt = np.ascontiguousarray(x.reshape(N, Dx))
    top4 = _route(xt, wg)

    # token lists per expert
    sel = np.zeros((N, E), dtype=bool)
    np.put_along_axis(sel, top4, True, axis=1)
    tokens = [np.nonzero(sel[:, e])[0] for e in range(E)]
    counts = np.array([len(t) for t in tokens])
    CAP = max(int(counts.max()), 256)
    CAP += CAP % 2

    if CAP not in _NC_CACHE:
        _NC_CACHE[CAP] = build_expert_kernel(CAP)
    nc = _NC_CACHE[CAP]

    xbf = xt.astype(NPBF16)
    in_maps = []
    for e in range(E):
        xe = np.zeros((CAP, Dx), dtype=NPBF16)
        xe[:counts[e]] = xbf[tokens[e]]
        in_maps.append({
            "xT": np.ascontiguousarray(xe.T),
            "w1r": _w1_slice_major(w1[e].astype(NPBF16)),
            "w2r": _w2_slice_major(w2[e].astype(NPBF16)),
        })

    res = bass_utils.run_bass_kernel_spmd(
        nc, in_maps, core_ids=list(range(E)), **(_run_kwargs or {})
    )

    out = np.zeros((N, O), dtype=np.float32)
    for e in range(E):
        out[tokens[e]] += res.results[e]["yT"].T[:counts[e]]
    out = out.reshape(B, S, O)
    if _want_results:
        return out, res
    return out
